# revision 9
# baseline (speedup 1.0000x reference)
"""Trainium2 Bass kernel for the ChimeraSurrogateNCA problem.

Masked 3x3 conv NCA, 5 steps, B=4 C=256 H=W=128, softsign residual.

Strategy: 8 cores = 2 batch-pairs x 4 horizontal quarters (each core: 2
batches, 32 owned rows + S-row halo each side; redundant halo compute,
zero inter-core comms). x is stored [cin -> 2x128 partition blocks,
(row, col) free] in fp16 with padded 132-wide rows so 3x3 shifts are
pure AP offsets; both cin blocks of a batch live in ONE tile so the
mask multiply fuses across them. The position-dependent causal mask is
applied on the DVE with host-pre-shifted, partition-broadcast mask
tiles, fused per dy-group (tap axis = stride-0 broadcast of the slab);
all 9 taps then PSUM-accumulate on the PE (fp16 matmuls, fp32
accumulation). The softsign residual spreads across engines: ACT does
|d| and +1 and the psum->fp16 copy, DVE does a one-instruction
approximate reciprocal (~51 ULP) and the slab add, GPSIMD does the
d*r multiply.
"""

import numpy as np

import concourse.bass as bass
import concourse.mybir as mybir
from concourse.tile import TileContext

F16 = mybir.dt.float16
F32 = mybir.dt.float32

N_CORES = 8
B, C, H, W = 4, 256, 128, 128
P = 128          # partitions / channel block size
CB = C // P      # channel blocks (2)
SW = 132         # padded slab row width; image col w <-> slab col w + 2
NB = 2           # batches per core
NQ = 4           # horizontal quarters
OWN = H // NQ    # rows owned per core (32)

# taps excluding the always-unmasked center (k=4), grouped by dy.
# kk (mask plane index) follows this order: taps 0,1,2,3,5,6,7,8.
DY_GROUPS = [(0, [0, 1, 2]), (1, [3, 5]), (2, [6, 7, 8])]
TAPS = [k for _, taps in DY_GROUPS for k in taps]
KK0 = {0: 0, 1: 3, 2: 5}  # index of each dy group's first tap in TAPS


def _build_program(S, hoist=True):
    SR = OWN + 2 * S  # slab rows
    nc = bass.Bass()
    xin = nc.declare_dram_parameter("xin", [NB, P, CB * SR * SW], F16, isOutput=False)
    mk = nc.declare_dram_parameter("mk", [P, 8 * SR * SW], F16, isOutput=False)
    wt = nc.declare_dram_parameter("wt", [CB, P, 9 * CB * P], F16, isOutput=False)
    out = nc.declare_dram_parameter("out", [NB, P, CB * OWN * W], F16, isOutput=True)

    with TileContext(nc) as tc:
        with (
            tc.tile_pool(name="xp", bufs=1) as xpool,
            tc.tile_pool(name="mp", bufs=1) as mpool,
            tc.tile_pool(name="wp", bufs=1) as wpool,
            tc.tile_pool(name="ap", bufs=2) as apool,
            tc.tile_pool(name="tp", bufs=3) as tpool,
            tc.tile_pool(name="pp", bufs=3, space="PSUM") as ppool,
        ):
            # Startup DMAs spread across engine queues so they run in
            # parallel: weights+slab0 on sync, slab1 on tensor, the three
            # per-dy-group mask chunks on scalar/vector/gpsimd.
            w_sb = []
            for cb in range(CB):
                t = wpool.tile([P, 9 * CB * P], F16, tag=f"w{cb}")
                nc.sync.dma_start(out=t[:], in_=wt[cb])
                w_sb.append(t)
            slab = {}
            for b, eng in ((0, nc.sync), (1, nc.sync)):
                t = xpool.tile([P, CB * SR * SW], F16, tag=f"slab{b}")
                eng.dma_start(out=t[:], in_=xin[b])
                slab[b] = t
            mkv = mk.rearrange("p (k f) -> p k f", k=8)
            mk_sb = {}
            for (dy, taps), eng in zip(DY_GROUPS, (nc.gpsimd, nc.scalar, nc.gpsimd)):
                T = len(taps)
                kk = KK0[dy]
                t = mpool.tile([P, T * SR * SW], F16, tag=f"mk{dy}")
                eng.dma_start(
                    out=t.rearrange("p (k f) -> p k f", k=T),
                    in_=mkv[:, kk:kk + T],
                )
                mk_sb[dy] = t

            def w_view(k, cb, ob):
                return w_sb[cb][:, (k * CB + ob) * P:(k * CB + ob + 1) * P]

            def slab_rows(b, cb, q0, R, c0, cw):
                v = slab[b].rearrange("p (c r w) -> p c r w", c=CB, w=SW)
                return v[:, cb, q0:q0 + R, c0:c0 + cw]

            # dy0 last: its abuild reads slab row r0-1, written by the
            # previous group's residual — emitting it last (and running its
            # matmuls last) gives the scheduler maximal slack to overlap
            # the cross-engine residual chain with independent PE work.
            DY_ORDER = [1, 2, 0]

            def emit_abuild(b, r0, R):
                # One DVE op per dy group covering both cin blocks and all
                # taps of the group: out[p, cb, t, R*SW] = slab[p, cb,
                # (rows)] (broadcast over t) * mask[p, t, (rows)]
                # (broadcast over cb).
                tiles = {}
                sv = slab[b].rearrange("p (c f) -> p c f", c=CB)
                for dy in DY_ORDER:
                    T = len(dict(DY_GROUPS)[dy])
                    q0 = r0 + dy - 1
                    at = apool.tile([P, CB * T * R * SW], F16, tag=f"a{dy}")
                    in0 = sv[:, :, q0 * SW:(q0 + R) * SW]          # [P, CB, R*SW]
                    in0 = in0.unsqueeze(2).broadcast_to([P, CB, T, R * SW])
                    mv = mk_sb[dy].rearrange("p (k f) -> p k f", k=T)
                    in1 = mv[:, :, q0 * SW:(q0 + R) * SW]          # [P, T, R*SW]
                    in1 = in1.unsqueeze(1).broadcast_to([P, CB, T, R * SW])
                    ov = at.rearrange(
                        "p (c t f) -> p c t f", c=CB, t=T
                    )
                    nc.vector.tensor_tensor(
                        out=ov, in0=in0, in1=in1, op=mybir.AluOpType.mult
                    )
                    tiles[dy] = at
                return tiles

            def emit_center(b, r0, R, psums):
                for ob in range(CB):
                    for cb in range(CB):
                        rhs = slab_rows(b, cb, r0, R, 2, W)
                        nc.tensor.matmul(
                            psums[ob][:], w_view(4, cb, ob), rhs,
                            start=(cb == 0), stop=False,
                        )

            def emit_rest(b, r0, R, tiles, psums):
                for ob in range(CB):
                    n = 0
                    for dy in DY_ORDER:
                        taps = dict(DY_GROUPS)[dy]
                        at = tiles[dy].rearrange(
                            "p (c t r w) -> p c t r w", c=CB, t=len(taps), w=SW
                        )
                        for ti, k in enumerate(taps):
                            dx = k % 3
                            for cb in range(CB):
                                n += 1
                                rhs = at[:, cb, ti, :, dx + 1:dx + 1 + W]
                                nc.tensor.matmul(
                                    psums[ob][:], w_view(k, cb, ob), rhs,
                                    start=False, stop=(n == 2 * len(TAPS)),
                                )

            def emit_resid(b, r0, R, psums):
                # x += d/(1+|d|) with 1/(1+|d|) = exp(-ln(|d|+1)): the
                # reciprocal runs as ACT LUT ops (ln/exp share one table
                # set with abs/copy), keeping the DVE to just the final
                # slab add.
                for ob in range(CB):
                    ps = psums[ob]
                    u = tpool.tile([P, R * W], F32, tag="u")
                    nc.scalar.activation(
                        out=u[:], in_=ps[:],
                        func=mybir.ActivationFunctionType.Abs,
                    )
                    lt = tpool.tile([P, R * W], F32, tag="lt")
                    nc.scalar.activation(
                        out=lt[:], in_=u[:],
                        func=mybir.ActivationFunctionType.Ln, bias=1.0,
                    )
                    rt = tpool.tile([P, R * W], F32, tag="rt")
                    nc.scalar.activation(
                        out=rt[:], in_=lt[:],
                        func=mybir.ActivationFunctionType.Exp, scale=-1.0,
                    )
                    dsb = tpool.tile([P, R * W], F16, tag="dsb")
                    nc.scalar.copy(out=dsb[:], in_=ps[:])
                    gt = tpool.tile([P, R * W], F16, tag="gt")
                    nc.gpsimd.tensor_tensor(
                        out=gt[:], in0=dsb[:], in1=rt[:], op=mybir.AluOpType.mult
                    )
                    sv = slab_rows(b, ob, r0, R, 2, W)
                    gv = gt.rearrange("p (r c) -> p r c", c=W)
                    nc.vector.tensor_tensor(
                        out=sv, in0=sv, in1=gv, op=mybir.AluOpType.add
                    )

            for t in range(1, S + 1):
                lo, hi = t, SR - t
                for b in range(NB):
                    groups = []
                    r = lo
                    while r < hi:
                        Rg = min(4, hi - r)
                        groups.append((r, Rg))
                        r += Rg
                    pending = None
                    for (r0, Rg) in groups:
                        tiles = emit_abuild(b, r0, Rg)
                        psums = [
                            ppool.tile([P, Rg * W], F32, tag=f"ps{ob}", name=f"ps{ob}")
                            for ob in range(CB)
                        ]
                        emit_center(b, r0, Rg, psums)
                        if pending is not None:
                            emit_resid(b, *pending)
                        emit_rest(b, r0, Rg, tiles, psums)
                        pending = (r0, Rg, psums)
                    emit_resid(b, *pending)

            # Store in row-halves, early rows first: their residuals finish
            # first, so the DMAs stream out while later groups compute.
            HO = OWN // 2
            for b in range(NB):
                ov = out[b].rearrange("p (c r w) -> p c r w", c=CB, w=W)
                for half in range(2):
                    for cb in range(CB):
                        nc.sync.dma_start(
                            out=ov[:, cb, half * HO:(half + 1) * HO],
                            in_=slab_rows(b, cb, S + half * HO, HO, 2, W),
                        )
    if hoist:
        _hoist_extra_waits(nc)
    return nc


# Engine compute instructions have a single hardware sync-wait slot on
# trn2 (walrus: "Too many sync wait commands"); Tile may attach 2-3.
# Hoist the extras onto standalone EventSemaphore waits on the same
# engine queue immediately before the instruction.
_NO_HOIST = {
    "InstEventSemaphore", "InstCall",
    "InstUnconditionalBranch", "InstRegisterMove",
}


def _hoist_extra_waits(nc, max_waits=1):
    fn = nc.m.functions[0]
    n = 0
    for blk in fn.blocks:
        newlist = []
        for inst in blk.instructions:
            if (
                type(inst).__name__ == "InstISA"
                and getattr(inst, "op_name", "") == "EVENT_SEMAPHORE_RANGE_CLEAR"
            ):
                # kernel-tail lazy-sem reset; this walrus can't encode
                # opcode 176 ("ISA wrong length"). Only needed for NEFF
                # re-execution, which the runtime handles via fresh loads.
                continue
            si = inst.sync_info
            if (
                si is not None
                and si.on_wait
                and len(si.on_wait) > max_waits
                and type(inst).__name__ not in _NO_HOIST
            ):
                waits = list(si.on_wait)
                extra, keep = waits[:-max_waits], waits[-max_waits:]
                for j, wsub in enumerate(extra):
                    carrier = mybir.InstEventSemaphore(
                        name=f"hwait-{inst.name}-{j}", ins=[], outs=[]
                    )
                    carrier.engine = inst.engine
                    carrier.sync_info = type(si)(on_wait=[wsub], on_update=[])
                    newlist.append(carrier)
                    n += 1
                inst.sync_info = type(si)(
                    on_wait=keep, on_update=list(si.on_update or [])
                )
            newlist.append(inst)
        try:
            blk.instructions = newlist
        except Exception:
            blk.instructions[:] = newlist
    return n


def _pack_weights(Wt):
    # wt[cb][p, k*2*P + ob*P + co] = Wt[ob*P + co, cb*P + p, k]
    Wr = np.ascontiguousarray(Wt.reshape(C, C, 9))
    wta = Wr.reshape(CB, P, CB, P, 9)            # [ob, co, cb, p, k]
    wta = wta.transpose(2, 3, 4, 0, 1)           # [cb, p, k, ob, co]
    return np.ascontiguousarray(wta.reshape(CB, P, 9 * CB * P)).astype(np.float16)


def _pack_core_inputs(core, S, retina, mask, wt_host):
    SR = OWN + 2 * S
    bg, q = divmod(core, NQ)   # batch group 0/1, quarter 0..3
    ir0 = q * OWN - S          # image row of slab row 0
    xin_host = np.zeros((NB, CB, P, SR, SW), np.float16)
    rlo = max(0, -ir0)
    rhi = min(SR, H - ir0)
    if rhi > rlo:
        xin_host[:, :, :, rlo:rhi, 2:2 + W] = retina.reshape(B, CB, P, H, W)[
            NB * bg:NB * (bg + 1), :, :, ir0 + rlo:ir0 + rhi, :
        ].astype(np.float16)
    mk_host = np.zeros((8, SR, SW), np.float32)
    for kk, k in enumerate(TAPS):
        dy, dx = k // 3, k % 3
        # M'[q, v] = mask[k, image_row(q - dy + 1), v - dx - 1]
        irow = ir0 + np.arange(SR) - dy + 1
        wcol = np.arange(SW) - dx - 1
        rr = np.where((irow >= 0) & (irow < H))[0]
        cc = np.where((wcol >= 0) & (wcol < W))[0]
        if len(rr) and len(cc):
            mk_host[kk][np.ix_(rr, cc)] = mask[k][irow[rr][:, None], wcol[cc][None, :]]
    mk_b = np.ascontiguousarray(
        np.broadcast_to(
            mk_host.reshape(1, 8 * SR * SW).astype(np.float16), (P, 8 * SR * SW)
        )
    )
    return {
        "xin": np.ascontiguousarray(
            xin_host.transpose(0, 2, 1, 3, 4)
        ).reshape(NB, P, CB * SR * SW),
        "mk": mk_b,
        "wt": wt_host,
    }


def make_in_maps(S, retina, evolve_weight, causal_mask):
    retina = np.asarray(retina, dtype=np.float32)
    Wt = np.asarray(evolve_weight, dtype=np.float32)
    mask = np.asarray(causal_mask, dtype=np.float32).reshape(9, H, W)
    wt_host = _pack_weights(Wt)
    return [_pack_core_inputs(i, S, retina, mask, wt_host) for i in range(N_CORES)]


def gather_output(results):
    outf = np.zeros((B, CB, P, H, W), np.float32)
    for core in range(N_CORES):
        bg, q = divmod(core, NQ)
        o = np.asarray(results[core]["out"]).reshape(NB, P, CB, OWN, W)
        outf[NB * bg:NB * (bg + 1), :, :, q * OWN:(q + 1) * OWN, :] = (
            o.transpose(0, 2, 1, 3, 4).astype(np.float32)
        )
    return outf.reshape(B, C, H, W)


def kernel(retina, evolve_weight, causal_mask, steps):
    from concourse.bass_utils import run_bass_kernel_spmd

    S = int(steps)
    if S <= 0:
        return np.asarray(retina, dtype=np.float32).copy()
    nc = _build_program(S)
    in_maps = make_in_maps(S, retina, evolve_weight, causal_mask)
    res = run_bass_kernel_spmd(nc, in_maps, list(range(N_CORES)))
    return gather_output(res.results)


# revision 14
# speedup vs baseline: 1.0356x; 1.0356x over previous
"""Trainium2 Bass kernel for the ChimeraSurrogateNCA problem.

Masked 3x3 conv NCA, 5 steps, B=4 C=256 H=W=128, softsign residual.

Strategy: 8 cores = 2 batch-pairs x 4 horizontal quarters (each core: 2
batches, 32 owned rows + S-row halo each side; redundant halo compute,
zero inter-core comms). x is stored [cin -> 2x128 partition blocks,
(row, col) free] in fp16 with padded 132-wide rows so 3x3 shifts are
pure AP offsets; both cin blocks of a batch live in ONE tile so the
mask multiply fuses across them. The position-dependent causal mask is
applied on the DVE with host-pre-shifted, partition-broadcast mask
tiles, fused per dy-group (tap axis = stride-0 broadcast of the slab);
all 9 taps then PSUM-accumulate on the PE (fp16 matmuls, fp32
accumulation). The softsign residual spreads across engines: ACT does
|d| and +1 and the psum->fp16 copy, DVE does a one-instruction
approximate reciprocal (~51 ULP) and the slab add, GPSIMD does the
d*r multiply.
"""

import numpy as np

import concourse.bass as bass
import concourse.mybir as mybir
from concourse.tile import TileContext

F16 = mybir.dt.float16
F32 = mybir.dt.float32

N_CORES = 8
B, C, H, W = 4, 256, 128, 128
P = 128          # partitions / channel block size
CB = C // P      # channel blocks (2)
SW = 132         # padded slab row width; image col w <-> slab col w + 2
NB = 2           # batches per core
NQ = 4           # horizontal quarters
OWN = H // NQ    # rows owned per core (32)

# taps excluding the always-unmasked center (k=4), grouped by dy.
# kk (mask plane index) follows this order: taps 0,1,2,3,5,6,7,8.
DY_GROUPS = [(0, [0, 1, 2]), (1, [3, 5]), (2, [6, 7, 8])]
TAPS = [k for _, taps in DY_GROUPS for k in taps]
KK0 = {0: 0, 1: 3, 2: 5}  # index of each dy group's first tap in TAPS


def _build_program(S, hoist=True):
    SR = OWN + 2 * S  # slab rows
    # mask row-chunk boundaries: first chunk small so the first groups'
    # abuild can start ~10us in; later chunks stream behind compute.
    MCH = [0, SR // 3, 2 * SR // 3, SR]
    nc = bass.Bass()
    xin = nc.declare_dram_parameter("xin", [NB, P, CB * SR * SW], F16, isOutput=False)
    mk = nc.declare_dram_parameter("mk", [P, 8 * SR * SW], F16, isOutput=False)
    wt = nc.declare_dram_parameter("wt", [CB, P, 9 * CB * P], F16, isOutput=False)
    out = nc.declare_dram_parameter("out", [NB, P, CB * OWN * SW], F16, isOutput=True)

    with TileContext(nc) as tc:
        with (
            tc.tile_pool(name="xp", bufs=1) as xpool,
            tc.tile_pool(name="mp", bufs=1) as mpool,
            tc.tile_pool(name="wp", bufs=1) as wpool,
            tc.tile_pool(name="ap", bufs=2) as apool,
            tc.tile_pool(name="tp", bufs=3) as tpool,
            tc.tile_pool(name="pp", bufs=3, space="PSUM") as ppool,
        ):
            # Startup DMAs: row-chunked and spread across the three DMA
            # queues (sync/scalar/gpsimd) so the first groups' inputs land
            # ~10us in and the rest streams behind compute.
            w_sb = []
            for cb in range(CB):
                t = wpool.tile([P, 9 * CB * P], F16, tag=f"w{cb}")
                nc.sync.dma_start(out=t[:], in_=wt[cb])
                w_sb.append(t)
            xinv = xin.rearrange("b p (c r w) -> b p c r w", c=CB, w=SW)
            slab = {}
            for b in range(NB):
                t = xpool.tile([P, CB * SR * SW], F16, tag=f"slab{b}")
                tv = t.rearrange("p (c r w) -> p c r w", c=CB, w=SW)
                for lo, hi in zip(MCH[:-1], MCH[1:]):
                    nc.sync.dma_start(
                        out=tv[:, :, lo:hi], in_=xinv[b, :, :, lo:hi]
                    )
                slab[b] = t
            mkv = mk.rearrange("p (k r w) -> p k r w", k=8, w=SW)
            mk_sb = {}
            for ci, (lo, hi) in enumerate(zip(MCH[:-1], MCH[1:])):
                for (dy, taps), eng in zip(
                    DY_GROUPS, (nc.scalar, nc.scalar, nc.gpsimd)
                ):
                    T = len(taps)
                    kk = KK0[dy]
                    t = mpool.tile(
                        [P, T * (hi - lo) * SW], F16, tag=f"mk{dy}c{ci}"
                    )
                    eng.dma_start(
                        out=t.rearrange("p (k r w) -> p k r w", k=T, w=SW),
                        in_=mkv[:, kk:kk + T, lo:hi],
                    )
                    mk_sb[dy, ci] = t

            def w_view(k, cb, ob):
                return w_sb[cb][:, (k * CB + ob) * P:(k * CB + ob + 1) * P]

            def slab_rows(b, cb, q0, R, c0, cw):
                v = slab[b].rearrange("p (c r w) -> p c r w", c=CB, w=SW)
                return v[:, cb, q0:q0 + R, c0:c0 + cw]

            # dy0 last: its abuild reads slab row r0-1, written by the
            # previous group's residual — emitting it last (and running its
            # matmuls last) gives the scheduler maximal slack to overlap
            # the cross-engine residual chain with independent PE work.
            DY_ORDER = [1, 2, 0]

            def emit_abuild(b, r0, R):
                # One DVE op per (dy group x mask row-chunk) covering both
                # cin blocks and all taps of the group: out[p, cb, t, rows]
                # = slab[p, cb, rows] (broadcast over t) * mask[p, t, rows]
                # (broadcast over cb). Ops split at mask-chunk boundaries.
                tiles = {}
                sv = slab[b].rearrange("p (c f) -> p c f", c=CB)
                for dy in DY_ORDER:
                    T = len(dict(DY_GROUPS)[dy])
                    q0 = r0 + dy - 1
                    at = apool.tile([P, CB * T * R * SW], F16, tag=f"a{dy}")
                    ov = at.rearrange("p (c t f) -> p c t f", c=CB, t=T)
                    for ci, (lo, hi) in enumerate(zip(MCH[:-1], MCH[1:])):
                        ra, rb = max(q0, lo), min(q0 + R, hi)
                        if ra >= rb:
                            continue
                        n = (rb - ra) * SW
                        in0 = sv[:, :, ra * SW:rb * SW]
                        in0 = in0.unsqueeze(2).broadcast_to([P, CB, T, n])
                        mv = mk_sb[dy, ci].rearrange("p (k f) -> p k f", k=T)
                        in1 = mv[:, :, (ra - lo) * SW:(rb - lo) * SW]
                        in1 = in1.unsqueeze(1).broadcast_to([P, CB, T, n])
                        o = ov[:, :, :, (ra - q0) * SW:(rb - q0) * SW]
                        nc.vector.tensor_tensor(
                            out=o, in0=in0, in1=in1, op=mybir.AluOpType.mult
                        )
                    tiles[dy] = at
                return tiles

            def emit_center(b, r0, R, psums):
                for ob in range(CB):
                    for cb in range(CB):
                        rhs = slab_rows(b, cb, r0, R, 2, W)
                        nc.tensor.matmul(
                            psums[ob][:], w_view(4, cb, ob), rhs,
                            start=(cb == 0), stop=False,
                        )

            def emit_rest(b, r0, R, tiles, psums):
                for ob in range(CB):
                    n = 0
                    for dy in DY_ORDER:
                        taps = dict(DY_GROUPS)[dy]
                        at = tiles[dy].rearrange(
                            "p (c t r w) -> p c t r w", c=CB, t=len(taps), w=SW
                        )
                        for ti, k in enumerate(taps):
                            dx = k % 3
                            for cb in range(CB):
                                n += 1
                                rhs = at[:, cb, ti, :, dx + 1:dx + 1 + W]
                                nc.tensor.matmul(
                                    psums[ob][:], w_view(k, cb, ob), rhs,
                                    start=False, stop=(n == 2 * len(TAPS)),
                                )

            def emit_resid(b, r0, R, psums):
                # x += d/(1+|d|) with 1/(1+|d|) = exp(-ln(|d|+1)): the
                # reciprocal runs as ACT LUT ops (ln/exp share one table
                # set with abs/copy), keeping the DVE to just the final
                # slab add.
                for ob in range(CB):
                    ps = psums[ob]
                    u = tpool.tile([P, R * W], F32, tag="u")
                    nc.scalar.activation(
                        out=u[:], in_=ps[:],
                        func=mybir.ActivationFunctionType.Abs,
                    )
                    lt = tpool.tile([P, R * W], F32, tag="lt")
                    nc.scalar.activation(
                        out=lt[:], in_=u[:],
                        func=mybir.ActivationFunctionType.Ln, bias=1.0,
                    )
                    rt = tpool.tile([P, R * W], F32, tag="rt")
                    nc.scalar.activation(
                        out=rt[:], in_=lt[:],
                        func=mybir.ActivationFunctionType.Exp, scale=-1.0,
                    )
                    dsb = tpool.tile([P, R * W], F16, tag="dsb")
                    nc.scalar.copy(out=dsb[:], in_=ps[:])
                    gt = tpool.tile([P, R * W], F16, tag="gt")
                    nc.gpsimd.tensor_tensor(
                        out=gt[:], in0=dsb[:], in1=rt[:], op=mybir.AluOpType.mult
                    )
                    sv = slab_rows(b, ob, r0, R, 2, W)
                    gv = gt.rearrange("p (r c) -> p r c", c=W)
                    nc.vector.tensor_tensor(
                        out=sv, in0=sv, in1=gv, op=mybir.AluOpType.add
                    )

            for t in range(1, S + 1):
                lo, hi = t, SR - t
                for b in range(NB):
                    groups = []
                    r = lo
                    while r < hi:
                        Rg = min(4, hi - r)
                        groups.append((r, Rg))
                        r += Rg
                    pending = None
                    for (r0, Rg) in groups:
                        tiles = emit_abuild(b, r0, Rg)
                        psums = [
                            ppool.tile([P, Rg * W], F32, tag=f"ps{ob}", name=f"ps{ob}")
                            for ob in range(CB)
                        ]
                        emit_center(b, r0, Rg, psums)
                        if pending is not None:
                            emit_resid(b, *pending)
                        emit_rest(b, r0, Rg, tiles, psums)
                        pending = (r0, Rg, psums)
                    emit_resid(b, *pending)

            # Store padded rows contiguously (2KB+ packets), in quarters,
            # early rows first: their residuals finish first, so the DMAs
            # stream out while later groups compute.
            HO = OWN // 4
            for b in range(NB):
                ov = out[b].rearrange("p (c r w) -> p c r w", c=CB, w=SW)
                sv = slab[b].rearrange("p (c r w) -> p c r w", c=CB, w=SW)
                for q in range(4):
                    for cb in range(CB):
                        nc.sync.dma_start(
                            out=ov[:, cb, q * HO:(q + 1) * HO],
                            in_=sv[:, cb, S + q * HO:S + (q + 1) * HO],
                        )
    if hoist:
        _hoist_extra_waits(nc)
    return nc


# Engine compute instructions have a single hardware sync-wait slot on
# trn2 (walrus: "Too many sync wait commands"); Tile may attach 2-3.
# Hoist the extras onto standalone EventSemaphore waits on the same
# engine queue immediately before the instruction.
_NO_HOIST = {
    "InstEventSemaphore", "InstCall",
    "InstUnconditionalBranch", "InstRegisterMove",
}


def _hoist_extra_waits(nc, max_waits=1):
    fn = nc.m.functions[0]
    n = 0
    for blk in fn.blocks:
        newlist = []
        for inst in blk.instructions:
            if (
                type(inst).__name__ == "InstISA"
                and getattr(inst, "op_name", "") == "EVENT_SEMAPHORE_RANGE_CLEAR"
            ):
                # kernel-tail lazy-sem reset; this walrus can't encode
                # opcode 176 ("ISA wrong length"). Only needed for NEFF
                # re-execution, which the runtime handles via fresh loads.
                continue
            si = inst.sync_info
            if (
                si is not None
                and si.on_wait
                and len(si.on_wait) > max_waits
                and type(inst).__name__ not in _NO_HOIST
            ):
                waits = list(si.on_wait)
                extra, keep = waits[:-max_waits], waits[-max_waits:]
                for j, wsub in enumerate(extra):
                    carrier = mybir.InstEventSemaphore(
                        name=f"hwait-{inst.name}-{j}", ins=[], outs=[]
                    )
                    carrier.engine = inst.engine
                    carrier.sync_info = type(si)(on_wait=[wsub], on_update=[])
                    newlist.append(carrier)
                    n += 1
                inst.sync_info = type(si)(
                    on_wait=keep, on_update=list(si.on_update or [])
                )
            newlist.append(inst)
        try:
            blk.instructions = newlist
        except Exception:
            blk.instructions[:] = newlist
    return n


def _pack_weights(Wt):
    # wt[cb][p, k*2*P + ob*P + co] = Wt[ob*P + co, cb*P + p, k]
    Wr = np.ascontiguousarray(Wt.reshape(C, C, 9))
    wta = Wr.reshape(CB, P, CB, P, 9)            # [ob, co, cb, p, k]
    wta = wta.transpose(2, 3, 4, 0, 1)           # [cb, p, k, ob, co]
    return np.ascontiguousarray(wta.reshape(CB, P, 9 * CB * P)).astype(np.float16)


def _pack_core_inputs(core, S, retina, mask, wt_host):
    SR = OWN + 2 * S
    bg, q = divmod(core, NQ)   # batch group 0/1, quarter 0..3
    ir0 = q * OWN - S          # image row of slab row 0
    xin_host = np.zeros((NB, CB, P, SR, SW), np.float16)
    rlo = max(0, -ir0)
    rhi = min(SR, H - ir0)
    if rhi > rlo:
        xin_host[:, :, :, rlo:rhi, 2:2 + W] = retina.reshape(B, CB, P, H, W)[
            NB * bg:NB * (bg + 1), :, :, ir0 + rlo:ir0 + rhi, :
        ].astype(np.float16)
    mk_host = np.zeros((8, SR, SW), np.float32)
    for kk, k in enumerate(TAPS):
        dy, dx = k // 3, k % 3
        # M'[q, v] = mask[k, image_row(q - dy + 1), v - dx - 1]
        irow = ir0 + np.arange(SR) - dy + 1
        wcol = np.arange(SW) - dx - 1
        rr = np.where((irow >= 0) & (irow < H))[0]
        cc = np.where((wcol >= 0) & (wcol < W))[0]
        if len(rr) and len(cc):
            mk_host[kk][np.ix_(rr, cc)] = mask[k][irow[rr][:, None], wcol[cc][None, :]]
    mk_b = np.ascontiguousarray(
        np.broadcast_to(
            mk_host.reshape(1, 8 * SR * SW).astype(np.float16), (P, 8 * SR * SW)
        )
    )
    return {
        "xin": np.ascontiguousarray(
            xin_host.transpose(0, 2, 1, 3, 4)
        ).reshape(NB, P, CB * SR * SW),
        "mk": mk_b,
        "wt": wt_host,
    }


def make_in_maps(S, retina, evolve_weight, causal_mask):
    retina = np.asarray(retina, dtype=np.float32)
    Wt = np.asarray(evolve_weight, dtype=np.float32)
    mask = np.asarray(causal_mask, dtype=np.float32).reshape(9, H, W)
    wt_host = _pack_weights(Wt)
    return [_pack_core_inputs(i, S, retina, mask, wt_host) for i in range(N_CORES)]


def gather_output(results):
    outf = np.zeros((B, CB, P, H, W), np.float32)
    for core in range(N_CORES):
        bg, q = divmod(core, NQ)
        o = np.asarray(results[core]["out"]).reshape(NB, P, CB, OWN, SW)
        outf[NB * bg:NB * (bg + 1), :, :, q * OWN:(q + 1) * OWN, :] = (
            o[:, :, :, :, 2:2 + W].transpose(0, 2, 1, 3, 4).astype(np.float32)
        )
    return outf.reshape(B, C, H, W)


def kernel(retina, evolve_weight, causal_mask, steps):
    from concourse.bass_utils import run_bass_kernel_spmd

    S = int(steps)
    if S <= 0:
        return np.asarray(retina, dtype=np.float32).copy()
    nc = _build_program(S)
    in_maps = make_in_maps(S, retina, evolve_weight, causal_mask)
    res = run_bass_kernel_spmd(nc, in_maps, list(range(N_CORES)))
    return gather_output(res.results)


# revision 17
# speedup vs baseline: 1.0469x; 1.0109x over previous
"""Trainium2 Bass kernel for the ChimeraSurrogateNCA problem.

Masked 3x3 conv NCA, 5 steps, B=4 C=256 H=W=128, softsign residual.

Strategy: 8 cores = 2 batch-pairs x 4 horizontal quarters (each core: 2
batches, 32 owned rows + S-row halo each side; redundant halo compute,
zero inter-core comms). x is stored [cin -> 2x128 partition blocks,
(row, col) free] in fp16 with padded 132-wide rows so 3x3 shifts are
pure AP offsets; both cin blocks of a batch live in ONE tile so the
mask multiply fuses across them. The position-dependent causal mask is
applied on the DVE with host-pre-shifted, partition-broadcast mask
tiles, fused per dy-group (tap axis = stride-0 broadcast of the slab);
all 9 taps then PSUM-accumulate on the PE (fp16 matmuls, fp32
accumulation). The softsign residual spreads across engines: ACT does
|d| and +1 and the psum->fp16 copy, DVE does a one-instruction
approximate reciprocal (~51 ULP) and the slab add, GPSIMD does the
d*r multiply.
"""

import numpy as np

import concourse.bass as bass
import concourse.mybir as mybir
from concourse.tile import TileContext

F16 = mybir.dt.float16
F32 = mybir.dt.float32

N_CORES = 8
B, C, H, W = 4, 256, 128, 128
P = 128          # partitions / channel block size
CB = C // P      # channel blocks (2)
SW = 132         # padded slab row width; image col w <-> slab col w + 2
NB = 2           # batches per core
NQ = 4           # horizontal quarters
OWN = H // NQ    # rows owned per core (32)

# taps excluding the always-unmasked center (k=4), grouped by dy.
# kk (mask plane index) follows this order: taps 0,1,2,3,5,6,7,8.
DY_GROUPS = [(0, [0, 1, 2]), (1, [3, 5]), (2, [6, 7, 8])]
TAPS = [k for _, taps in DY_GROUPS for k in taps]
KK0 = {0: 0, 1: 3, 2: 5}  # index of each dy group's first tap in TAPS


def _build_program(S, hoist=True):
    SR = OWN + 2 * S  # slab rows
    # mask row-chunk boundaries: first chunk small so the first groups'
    # abuild can start ~10us in; later chunks stream behind compute.
    MCH = [0, 8, 18, 30, SR] if SR > 34 else [0, SR // 2, SR]
    nc = bass.Bass()
    xin = nc.declare_dram_parameter("xin", [NB, P, CB * SR * SW], F16, isOutput=False)
    mk = nc.declare_dram_parameter("mk", [P, 8 * SR * SW], F16, isOutput=False)
    wt = nc.declare_dram_parameter("wt", [CB, P, 9 * CB * P], F16, isOutput=False)
    out = nc.declare_dram_parameter("out", [NB, P, CB * OWN * SW], F16, isOutput=True)

    with TileContext(nc) as tc:
        with (
            tc.tile_pool(name="xp", bufs=1) as xpool,
            tc.tile_pool(name="mp", bufs=1) as mpool,
            tc.tile_pool(name="wp", bufs=1) as wpool,
            tc.tile_pool(name="ap", bufs=2) as apool,
            tc.tile_pool(name="tp", bufs=3) as tpool,
            tc.tile_pool(name="pp", bufs=3, space="PSUM") as ppool,
        ):
            # Startup DMAs: row-chunked and spread across the three DMA
            # queues (sync/scalar/gpsimd) so the first groups' inputs land
            # ~10us in and the rest streams behind compute.
            w_sb = []
            for cb in range(CB):
                t = wpool.tile([P, 9 * CB * P], F16, tag=f"w{cb}")
                nc.sync.dma_start(out=t[:], in_=wt[cb])
                w_sb.append(t)
            xinv = xin.rearrange("b p (c r w) -> b p c r w", c=CB, w=SW)
            slab = {}
            for b in range(NB):
                t = xpool.tile([P, CB * SR * SW], F16, tag=f"slab{b}")
                tv = t.rearrange("p (c r w) -> p c r w", c=CB, w=SW)
                for lo, hi in zip(MCH[:-1], MCH[1:]):
                    nc.sync.dma_start(
                        out=tv[:, :, lo:hi], in_=xinv[b, :, :, lo:hi]
                    )
                slab[b] = t
            mkv = mk.rearrange("p (k r w) -> p k r w", k=8, w=SW)
            mk_sb = {}
            for ci, (lo, hi) in enumerate(zip(MCH[:-1], MCH[1:])):
                # dy1 first on its queue — abuild consumes dy1 first.
                for dy, eng in ((1, nc.scalar), (0, nc.scalar), (2, nc.gpsimd)):
                    taps = dict(DY_GROUPS)[dy]
                    T = len(taps)
                    kk = KK0[dy]
                    t = mpool.tile(
                        [P, T * (hi - lo) * SW], F16, tag=f"mk{dy}c{ci}"
                    )
                    eng.dma_start(
                        out=t.rearrange("p (k r w) -> p k r w", k=T, w=SW),
                        in_=mkv[:, kk:kk + T, lo:hi],
                    )
                    mk_sb[dy, ci] = t

            def w_view(k, cb, ob):
                return w_sb[cb][:, (k * CB + ob) * P:(k * CB + ob + 1) * P]

            def slab_rows(b, cb, q0, R, c0, cw):
                v = slab[b].rearrange("p (c r w) -> p c r w", c=CB, w=SW)
                return v[:, cb, q0:q0 + R, c0:c0 + cw]

            # dy0 last: its abuild reads slab row r0-1, written by the
            # previous group's residual — emitting it last (and running its
            # matmuls last) gives the scheduler maximal slack to overlap
            # the cross-engine residual chain with independent PE work.
            DY_ORDER = [1, 2, 0]

            def emit_abuild(b, r0, R):
                # One DVE op per (dy group x mask row-chunk) covering both
                # cin blocks and all taps of the group: out[p, cb, t, rows]
                # = slab[p, cb, rows] (broadcast over t) * mask[p, t, rows]
                # (broadcast over cb). Ops split at mask-chunk boundaries.
                tiles = {}
                sv = slab[b].rearrange("p (c f) -> p c f", c=CB)
                for dy in DY_ORDER:
                    T = len(dict(DY_GROUPS)[dy])
                    q0 = r0 + dy - 1
                    at = apool.tile([P, CB * T * R * SW], F16, tag=f"a{dy}")
                    ov = at.rearrange("p (c t f) -> p c t f", c=CB, t=T)
                    for ci, (lo, hi) in enumerate(zip(MCH[:-1], MCH[1:])):
                        ra, rb = max(q0, lo), min(q0 + R, hi)
                        if ra >= rb:
                            continue
                        n = (rb - ra) * SW
                        in0 = sv[:, :, ra * SW:rb * SW]
                        in0 = in0.unsqueeze(2).broadcast_to([P, CB, T, n])
                        mv = mk_sb[dy, ci].rearrange("p (k f) -> p k f", k=T)
                        in1 = mv[:, :, (ra - lo) * SW:(rb - lo) * SW]
                        in1 = in1.unsqueeze(1).broadcast_to([P, CB, T, n])
                        o = ov[:, :, :, (ra - q0) * SW:(rb - q0) * SW]
                        nc.vector.tensor_tensor(
                            out=o, in0=in0, in1=in1, op=mybir.AluOpType.mult
                        )
                    tiles[dy] = at
                return tiles

            def emit_center(b, r0, R, psums):
                for ob in range(CB):
                    for cb in range(CB):
                        rhs = slab_rows(b, cb, r0, R, 2, W)
                        nc.tensor.matmul(
                            psums[ob][:], w_view(4, cb, ob), rhs,
                            start=(cb == 0), stop=False,
                        )

            def emit_rest(b, r0, R, tiles, psums):
                for ob in range(CB):
                    n = 0
                    for dy in DY_ORDER:
                        taps = dict(DY_GROUPS)[dy]
                        at = tiles[dy].rearrange(
                            "p (c t r w) -> p c t r w", c=CB, t=len(taps), w=SW
                        )
                        for ti, k in enumerate(taps):
                            dx = k % 3
                            for cb in range(CB):
                                n += 1
                                rhs = at[:, cb, ti, :, dx + 1:dx + 1 + W]
                                nc.tensor.matmul(
                                    psums[ob][:], w_view(k, cb, ob), rhs,
                                    start=False, stop=(n == 2 * len(TAPS)),
                                )

            def emit_resid(b, r0, R, psums):
                # x += d/(1+|d|) with 1/(1+|d|) = exp(-ln(|d|+1)): the
                # reciprocal runs as ACT LUT ops (ln/exp share one table
                # set with abs/copy), keeping the DVE to just the final
                # slab add.
                for ob in range(CB):
                    ps = psums[ob]
                    u = tpool.tile([P, R * W], F32, tag="u")
                    nc.scalar.activation(
                        out=u[:], in_=ps[:],
                        func=mybir.ActivationFunctionType.Abs,
                    )
                    lt = tpool.tile([P, R * W], F32, tag="lt")
                    nc.scalar.activation(
                        out=lt[:], in_=u[:],
                        func=mybir.ActivationFunctionType.Ln, bias=1.0,
                    )
                    rt = tpool.tile([P, R * W], F32, tag="rt")
                    nc.scalar.activation(
                        out=rt[:], in_=lt[:],
                        func=mybir.ActivationFunctionType.Exp, scale=-1.0,
                    )
                    dsb = tpool.tile([P, R * W], F16, tag="dsb")
                    nc.scalar.copy(out=dsb[:], in_=ps[:])
                    gt = tpool.tile([P, R * W], F16, tag="gt")
                    nc.gpsimd.tensor_tensor(
                        out=gt[:], in0=dsb[:], in1=rt[:], op=mybir.AluOpType.mult
                    )
                    sv = slab_rows(b, ob, r0, R, 2, W)
                    gv = gt.rearrange("p (r c) -> p r c", c=W)
                    nc.vector.tensor_tensor(
                        out=sv, in0=sv, in1=gv, op=mybir.AluOpType.add
                    )

            for t in range(1, S + 1):
                lo, hi = t, SR - t
                for b in range(NB):
                    groups = []
                    r = lo
                    while r < hi:
                        Rg = min(4, hi - r)
                        groups.append((r, Rg))
                        r += Rg
                    pending = None
                    for (r0, Rg) in groups:
                        tiles = emit_abuild(b, r0, Rg)
                        psums = [
                            ppool.tile([P, Rg * W], F32, tag=f"ps{ob}", name=f"ps{ob}")
                            for ob in range(CB)
                        ]
                        emit_center(b, r0, Rg, psums)
                        if pending is not None:
                            emit_resid(b, *pending)
                        emit_rest(b, r0, Rg, tiles, psums)
                        pending = (r0, Rg, psums)
                    emit_resid(b, *pending)

            # Store padded rows contiguously (2KB+ packets), in quarters,
            # early rows first and spread over all three DMA queues: the
            # stores stream out while later groups compute.
            HO = OWN // 4
            oeng = [nc.sync, nc.scalar, nc.gpsimd]
            n = 0
            for b in range(NB):
                ov = out[b].rearrange("p (c r w) -> p c r w", c=CB, w=SW)
                sv = slab[b].rearrange("p (c r w) -> p c r w", c=CB, w=SW)
                for q in range(4):
                    for cb in range(CB):
                        oeng[n % 3].dma_start(
                            out=ov[:, cb, q * HO:(q + 1) * HO],
                            in_=sv[:, cb, S + q * HO:S + (q + 1) * HO],
                        )
                        n += 1
    if hoist:
        _hoist_extra_waits(nc)
    return nc


# Engine compute instructions have a single hardware sync-wait slot on
# trn2 (walrus: "Too many sync wait commands"); Tile may attach 2-3.
# Hoist the extras onto standalone EventSemaphore waits on the same
# engine queue immediately before the instruction.
_NO_HOIST = {
    "InstEventSemaphore", "InstCall",
    "InstUnconditionalBranch", "InstRegisterMove",
}


def _hoist_extra_waits(nc, max_waits=1):
    fn = nc.m.functions[0]
    n = 0
    for blk in fn.blocks:
        newlist = []
        for inst in blk.instructions:
            if (
                type(inst).__name__ == "InstISA"
                and getattr(inst, "op_name", "") == "EVENT_SEMAPHORE_RANGE_CLEAR"
            ):
                # kernel-tail lazy-sem reset; this walrus can't encode
                # opcode 176 ("ISA wrong length"). Only needed for NEFF
                # re-execution, which the runtime handles via fresh loads.
                continue
            si = inst.sync_info
            if (
                si is not None
                and si.on_wait
                and len(si.on_wait) > max_waits
                and type(inst).__name__ not in _NO_HOIST
            ):
                waits = list(si.on_wait)
                extra, keep = waits[:-max_waits], waits[-max_waits:]
                for j, wsub in enumerate(extra):
                    carrier = mybir.InstEventSemaphore(
                        name=f"hwait-{inst.name}-{j}", ins=[], outs=[]
                    )
                    carrier.engine = inst.engine
                    carrier.sync_info = type(si)(on_wait=[wsub], on_update=[])
                    newlist.append(carrier)
                    n += 1
                inst.sync_info = type(si)(
                    on_wait=keep, on_update=list(si.on_update or [])
                )
            newlist.append(inst)
        try:
            blk.instructions = newlist
        except Exception:
            blk.instructions[:] = newlist
    return n


def _pack_weights(Wt):
    # wt[cb][p, k*2*P + ob*P + co] = Wt[ob*P + co, cb*P + p, k]
    Wr = np.ascontiguousarray(Wt.reshape(C, C, 9))
    wta = Wr.reshape(CB, P, CB, P, 9)            # [ob, co, cb, p, k]
    wta = wta.transpose(2, 3, 4, 0, 1)           # [cb, p, k, ob, co]
    return np.ascontiguousarray(wta.reshape(CB, P, 9 * CB * P)).astype(np.float16)


def _pack_core_inputs(core, S, retina, mask, wt_host):
    SR = OWN + 2 * S
    bg, q = divmod(core, NQ)   # batch group 0/1, quarter 0..3
    ir0 = q * OWN - S          # image row of slab row 0
    xin_host = np.zeros((NB, CB, P, SR, SW), np.float16)
    rlo = max(0, -ir0)
    rhi = min(SR, H - ir0)
    if rhi > rlo:
        xin_host[:, :, :, rlo:rhi, 2:2 + W] = retina.reshape(B, CB, P, H, W)[
            NB * bg:NB * (bg + 1), :, :, ir0 + rlo:ir0 + rhi, :
        ].astype(np.float16)
    mk_host = np.zeros((8, SR, SW), np.float32)
    for kk, k in enumerate(TAPS):
        dy, dx = k // 3, k % 3
        # M'[q, v] = mask[k, image_row(q - dy + 1), v - dx - 1]
        irow = ir0 + np.arange(SR) - dy + 1
        wcol = np.arange(SW) - dx - 1
        rr = np.where((irow >= 0) & (irow < H))[0]
        cc = np.where((wcol >= 0) & (wcol < W))[0]
        if len(rr) and len(cc):
            mk_host[kk][np.ix_(rr, cc)] = mask[k][irow[rr][:, None], wcol[cc][None, :]]
    mk_b = np.ascontiguousarray(
        np.broadcast_to(
            mk_host.reshape(1, 8 * SR * SW).astype(np.float16), (P, 8 * SR * SW)
        )
    )
    return {
        "xin": np.ascontiguousarray(
            xin_host.transpose(0, 2, 1, 3, 4)
        ).reshape(NB, P, CB * SR * SW),
        "mk": mk_b,
        "wt": wt_host,
    }


def make_in_maps(S, retina, evolve_weight, causal_mask):
    retina = np.asarray(retina, dtype=np.float32)
    Wt = np.asarray(evolve_weight, dtype=np.float32)
    mask = np.asarray(causal_mask, dtype=np.float32).reshape(9, H, W)
    wt_host = _pack_weights(Wt)
    return [_pack_core_inputs(i, S, retina, mask, wt_host) for i in range(N_CORES)]


def gather_output(results):
    outf = np.zeros((B, CB, P, H, W), np.float32)
    for core in range(N_CORES):
        bg, q = divmod(core, NQ)
        o = np.asarray(results[core]["out"]).reshape(NB, P, CB, OWN, SW)
        outf[NB * bg:NB * (bg + 1), :, :, q * OWN:(q + 1) * OWN, :] = (
            o[:, :, :, :, 2:2 + W].transpose(0, 2, 1, 3, 4).astype(np.float32)
        )
    return outf.reshape(B, C, H, W)


def kernel(retina, evolve_weight, causal_mask, steps):
    from concourse.bass_utils import run_bass_kernel_spmd

    S = int(steps)
    if S <= 0:
        return np.asarray(retina, dtype=np.float32).copy()
    nc = _build_program(S)
    in_maps = make_in_maps(S, retina, evolve_weight, causal_mask)
    res = run_bass_kernel_spmd(nc, in_maps, list(range(N_CORES)))
    return gather_output(res.results)
